# revision 1
# baseline (speedup 1.0000x reference)
"""Trainium2 Bass kernel for nn_RandomDelayGwAC (gnn_message_passing).

Strategy:
  - 256 independent replays ("runs"), one per start node; run i's result only
    needs pred[i] at the end (diagonal extraction).
  - The heap-replay schedule (proc/parent/valid) is a kernel *input*, so all
    gather/scatter indices are host-computable. Steps within a run form a
    dependency DAG via (a) parent message links and (b) same-node state
    chains; its depth is ~110 << 1280. We batch all DAG-level-l steps of all
    runs into one "round": a [B,384]x[384,256] + [B,384]x[384,128] matmul
    pair with indirect-DMA gathers from step-indexed HBM tables.
  - Data parallel over 8 NeuronCores: 32 runs per core.
  - Encoder (x@We.T+be) and decoder (diag@Wd.T+bd, log_softmax) run on host
    (trivial FLOPs); the device executes only the sequential message passing.
"""

import numpy as np

N = 256
T = 1280
H = 256
MSG = 128
NCORES = 8
RUNS_PER_CORE = N // NCORES
P = 128  # partitions / block size


# ----------------------------------------------------------------------------
# host-side scheduling
# ----------------------------------------------------------------------------

def _build_schedule(proc, parent, valid):
    """Compute DAG levels and gather links for every valid step.

    Returns a dict with everything the device program + input builder needs.
    """
    n, t_max = proc.shape
    ar = np.arange(n)

    lev = np.zeros((n, t_max), np.int32)
    prev_item = np.full((n, t_max), -1, np.int64)  # linear id r*T+t of prev same-node step
    last_lev = np.zeros((n, N), np.int32)
    last_item = np.full((n, N), -1, np.int64)

    for t in range(t_max):
        v = valid[:, t]
        node = proc[:, t]
        p = parent[:, t]
        lp = np.where(p >= 0, lev[ar, np.maximum(p, 0)], 0)
        ln = last_lev[ar, node]
        l = 1 + np.maximum(lp, ln)
        lev[:, t] = np.where(v, l, 0)
        prev_item[:, t] = np.where(v, last_item[ar, node], -1)
        last_lev[ar, node] = np.where(v, l, last_lev[ar, node])
        last_item[ar, node] = np.where(v, ar * t_max + t, last_item[ar, node])

    fin_item = last_item[ar, ar]  # last valid step processing node i in run i (or -1)

    depths = lev.max(axis=1)
    R = int(depths.max())

    # snake assignment of runs to cores by depth (balances level widths)
    order = np.argsort(-depths, kind="stable")
    core_of = np.zeros(n, np.int32)
    for k, r in enumerate(order):
        c = k % (2 * NCORES)
        core_of[r] = c if c < NCORES else 2 * NCORES - 1 - c
    runs_of = [np.where(core_of == c)[0] for c in range(NCORES)]

    # per-core per-level widths -> common block structure
    Wd = np.zeros((NCORES, R + 1), np.int64)
    for c in range(NCORES):
        sub = lev[runs_of[c]]
        Wd[c] = np.bincount(sub[sub > 0].ravel() * 0 + sub[sub > 0].ravel(), minlength=R + 1) if False else \
            np.bincount(sub[valid[runs_of[c]]], minlength=R + 1)
    maxw = Wd[:, 1:].max(axis=0)  # [R]
    nblk = np.maximum(1, np.ceil(maxw / P).astype(np.int64))  # blocks per round
    blk_base = np.concatenate([[0], np.cumsum(nblk)])  # [R+1]
    NBLK = int(blk_base[-1])
    TAB0 = NBLK * P  # first extra row (pred0 block / fmsg row)

    # slot assignment per core: global_slot[r*T+t]
    gslot = np.full(n * t_max, -1, np.int64)
    sidx = np.full((NCORES, P, NBLK), TAB0, np.int32)   # default: pred0 node 0
    midx = np.full((NCORES, P, NBLK), TAB0, np.int32)   # default: fmsg
    fidx = np.zeros((NCORES, P, 1), np.int32)

    lin = np.arange(t_max, dtype=np.int64)
    for c in range(NCORES):
        rs = runs_of[c]
        levc = lev[rs]          # [32, T]
        vmask = valid[rs]
        items_lev = levc[vmask]                       # flat item levels
        items_lin = (rs[:, None] * t_max + lin[None, :])[vmask]
        order2 = np.argsort(items_lev, kind="stable")
        lv_sorted = items_lev[order2]
        # rank within level
        starts = np.searchsorted(lv_sorted, np.arange(1, R + 2))
        # slot of k-th item (sorted) = k - level_start + 0
        ranks = np.arange(len(lv_sorted)) - starts[lv_sorted - 1 - 0]  # level l starts at starts[l-1]
        gs = (blk_base[lv_sorted - 1] * P) + ranks
        gslot[items_lin[order2]] = gs

    # gather indices
    proc_f = proc.ravel()
    parent_f = parent.ravel()
    prev_f = prev_item.ravel()
    valid_f = valid.ravel()
    for c in range(NCORES):
        rs = runs_of[c]
        for ridx in range(len(rs)):
            r = rs[ridx]
            base = r * t_max
            tt = np.where(valid_f[base:base + t_max])[0]
            gitems = gslot[base + tt]
            pv = prev_f[base + tt]
            s = np.where(pv >= 0, gslot[np.maximum(pv, 0)], TAB0 + proc_f[base + tt])
            pa = parent_f[base + tt]
            m = np.where(pa >= 0, gslot[base + np.maximum(pa, 0)], TAB0)
            blkp = gitems // P
            pp = gitems % P
            sidx[c, pp, blkp] = s
            midx[c, pp, blkp] = m
            fi = fin_item[r]
            fidx[c, ridx, 0] = gslot[fi] if fi >= 0 else TAB0 + r

    return {
        "R": R,
        "nblk": nblk,
        "blk_base": blk_base,
        "NBLK": NBLK,
        "TAB0": TAB0,
        "runs_of": runs_of,
        "sidx": sidx,
        "midx": midx,
        "fidx": fidx,
    }


# ----------------------------------------------------------------------------
# device program
# ----------------------------------------------------------------------------

_PROGRAM_CACHE = {}


def _build_program(nblk_per_round, NBLK):
    import concourse.bass as bass
    import concourse.mybir as mybir
    import concourse.tile as tile
    from concourse import bacc
    from concourse.masks import make_identity

    f32 = mybir.dt.float32
    f32r = mybir.dt.float32r
    i32 = mybir.dt.int32

    nc = bacc.Bacc("TRN2", target_bir_lowering=False, debug=False,
                   enable_asserts=False)

    NSROWS = NBLK * P + N       # outputs + pred0 block
    MSROWS = NBLK * P + 1       # outputs + fmsg

    pred0_d = nc.dram_tensor("pred0", (N, H), f32r, kind="ExternalInput")
    fmsg_d = nc.dram_tensor("fmsg", (1, MSG), f32r, kind="ExternalInput")
    ws_d = nc.dram_tensor("wsT", (P, 3 * H), f32r, kind="ExternalInput")
    wm_d = nc.dram_tensor("wmT", (P, 3 * H), f32r, kind="ExternalInput")
    bs_d = nc.dram_tensor("bsrow", (1, H), f32, kind="ExternalInput")
    bm_d = nc.dram_tensor("bmrow", (1, MSG), f32, kind="ExternalInput")
    sidx_d = nc.dram_tensor("sidx", (P, NBLK), i32, kind="ExternalInput")
    midx_d = nc.dram_tensor("midx", (P, NBLK), i32, kind="ExternalInput")
    fidx_d = nc.dram_tensor("fidx", (P, 1), i32, kind="ExternalInput")
    diag_d = nc.dram_tensor("diag", (P, H), f32, kind="ExternalOutput")

    ns_tab = nc.dram_tensor("ns_tab", (NSROWS, H), f32r, kind="Internal")
    msg_tab = nc.dram_tensor("msg_tab", (MSROWS, MSG), f32r, kind="Internal")

    with tile.TileContext(nc) as tc:
        with (
            tc.tile_pool(name="const", bufs=1) as cpool,
            tc.tile_pool(name="work", bufs=4) as wpool,
            tc.tile_pool(name="psA", bufs=2, space="PSUM") as psA,
            tc.tile_pool(name="psB", bufs=2, space="PSUM") as psB,
        ):
            ident_f = cpool.tile([P, P], f32)
            make_identity(nc, ident_f[:])
            ident = cpool.tile([P, P], f32r)
            nc.vector.tensor_copy(ident[:], ident_f[:])

            ws_sb = cpool.tile([P, 3 * H], f32r)
            nc.sync.dma_start(ws_sb[:], ws_d.ap()[:])
            wm_sb = cpool.tile([P, 3 * H], f32r)
            nc.sync.dma_start(wm_sb[:], wm_d.ap()[:])
            bs_bc = cpool.tile([P, H], f32)
            nc.sync.dma_start(bs_bc[:], bs_d.ap()[:].to_broadcast([P, H]))
            bm_bc = cpool.tile([P, MSG], f32)
            nc.sync.dma_start(bm_bc[:], bm_d.ap()[:].to_broadcast([P, MSG]))

            sidx_sb = cpool.tile([P, NBLK], i32)
            nc.sync.dma_start(sidx_sb[:], sidx_d.ap()[:])
            midx_sb = cpool.tile([P, NBLK], i32)
            nc.sync.dma_start(midx_sb[:], midx_d.ap()[:])
            fidx_sb = cpool.tile([P, 1], i32)
            nc.sync.dma_start(fidx_sb[:], fidx_d.ap()[:])

            # init tables: pred0 rows and fmsg row
            for g in range(N // P):
                stage = wpool.tile([P, H], f32r, tag="stage")
                nc.sync.dma_start(stage[:], pred0_d.ap()[g * P:(g + 1) * P, :])
                nc.sync.dma_start(
                    ns_tab.ap()[NBLK * P + g * P: NBLK * P + (g + 1) * P, :],
                    stage[:])
            fstage = wpool.tile([1, MSG], f32r, tag="fstage")
            nc.sync.dma_start(fstage[:], fmsg_d.ap()[:])
            nc.sync.dma_start(msg_tab.ap()[NBLK * P: NBLK * P + 1, :], fstage[:])

            relu = mybir.ActivationFunctionType.Relu
            ident_act = mybir.ActivationFunctionType.Identity

            blk = 0
            for l, nb in enumerate(nblk_per_round):
                for j in range(int(nb)):
                    in_sb = wpool.tile([P, 3 * P], f32r, tag="in_sb")
                    nc.gpsimd.indirect_dma_start(
                        out=in_sb[:, 0:H], out_offset=None,
                        in_=ns_tab.ap()[:],
                        in_offset=bass.IndirectOffsetOnAxis(
                            ap=sidx_sb[:, blk:blk + 1], axis=0),
                    )
                    nc.gpsimd.indirect_dma_start(
                        out=in_sb[:, H:H + MSG], out_offset=None,
                        in_=msg_tab.ap()[:],
                        in_offset=bass.IndirectOffsetOnAxis(
                            ap=midx_sb[:, blk:blk + 1], axis=0),
                    )

                    inT_ps = psA.tile([P, 3 * P], f32r, tag="inT_ps", space="PSUM")
                    for k in range(3):
                        nc.tensor.transpose(
                            out=inT_ps[:, k * P:(k + 1) * P],
                            in_=in_sb[:, k * P:(k + 1) * P],
                            identity=ident[:])
                    inT = wpool.tile([P, 3 * P], f32r, tag="inT")
                    nc.vector.tensor_copy(inT[:], inT_ps[:])

                    ns_ps = psA.tile([P, H], f32, tag="ns_ps", space="PSUM")
                    for k in range(3):
                        nc.tensor.matmul(
                            ns_ps[:],
                            lhsT=inT[:, k * P:(k + 1) * P],
                            rhs=ws_sb[:, k * H:(k + 1) * H],
                            start=(k == 0), stop=(k == 2))
                    nc.vector.tensor_add(ns_ps[:], ns_ps[:], bs_bc[:])
                    ns_sb = wpool.tile([P, H], f32r, tag="ns_sb")
                    nc.scalar.activation(ns_sb[:], ns_ps[:], relu)

                    nsT_ps = psB.tile([P, H], f32r, tag="nsT_ps", space="PSUM")
                    for k in range(2):
                        nc.tensor.transpose(
                            out=nsT_ps[:, k * P:(k + 1) * P],
                            in_=ns_sb[:, k * P:(k + 1) * P],
                            identity=ident[:])
                    nsT = wpool.tile([P, H], f32r, tag="nsT")
                    nc.vector.tensor_copy(nsT[:], nsT_ps[:])

                    nm_ps = psB.tile([P, H], f32, tag="nm_ps", space="PSUM")
                    for k in range(3):
                        lhsT = nsT[:, k * P:(k + 1) * P] if k < 2 else inT[:, 2 * P:3 * P]
                        nc.tensor.matmul(
                            nm_ps[:], lhsT=lhsT,
                            rhs=wm_sb[:, k * H:(k + 1) * H],
                            start=(k == 0), stop=(k == 2))
                    nc.vector.tensor_add(nm_ps[:, 0:MSG], nm_ps[:, 0:MSG], bm_bc[:])
                    nm_sb = wpool.tile([P, MSG], f32r, tag="nm_sb")
                    nc.scalar.activation(nm_sb[:], nm_ps[:, 0:MSG], ident_act)

                    nc.sync.dma_start(ns_tab.ap()[blk * P:(blk + 1) * P, :], ns_sb[:])
                    nc.sync.dma_start(msg_tab.ap()[blk * P:(blk + 1) * P, :], nm_sb[:])
                    blk += 1

            # final diagonal extraction
            dout = wpool.tile([P, H], f32r, tag="dout")
            nc.gpsimd.indirect_dma_start(
                out=dout[:], out_offset=None,
                in_=ns_tab.ap()[:],
                in_offset=bass.IndirectOffsetOnAxis(ap=fidx_sb[:, :1], axis=0),
            )
            nc.sync.dma_start(diag_d.ap()[:], dout[:].bitcast(f32))

    nc.compile()
    return nc


# ----------------------------------------------------------------------------
# entry point
# ----------------------------------------------------------------------------

def kernel(x, first_message, We, be, Ws, bs, Wm, bm, Wd, bd, proc, parent, valid):
    from concourse import bass_utils

    x = np.asarray(x, np.float32)
    first_message = np.asarray(first_message, np.float32)
    We = np.asarray(We, np.float32)
    be = np.asarray(be, np.float32)
    Ws = np.asarray(Ws, np.float32)
    bs = np.asarray(bs, np.float32)
    Wm = np.asarray(Wm, np.float32)
    bm = np.asarray(bm, np.float32)
    Wd = np.asarray(Wd, np.float32)
    bd = np.asarray(bd, np.float32)
    proc = np.asarray(proc, np.int32)
    parent = np.asarray(parent, np.int32)
    valid = np.asarray(valid, bool)

    key = (proc.tobytes(), parent.tobytes(), valid.tobytes())
    import hashlib
    h = hashlib.sha1()
    for k in key:
        h.update(k)
    skey = h.hexdigest()
    if skey in _PROGRAM_CACHE:
        sched, nc = _PROGRAM_CACHE[skey]
    else:
        sched = _build_schedule(proc, parent, valid)
        nc = _build_program(sched["nblk"], sched["NBLK"])
        _PROGRAM_CACHE[skey] = (sched, nc)

    pred0 = (x @ We.T + be).astype(np.float32)

    # weight tiles: [K=384, N] -> [128, 3*N] chunk layout
    WsT = np.ascontiguousarray(
        Ws.T.reshape(3, P, H).transpose(1, 0, 2).reshape(P, 3 * H)).astype(np.float32)
    WmTp = np.zeros((3 * P, H), np.float32)
    WmTp[:, :MSG] = Wm.T
    WmT = np.ascontiguousarray(
        WmTp.reshape(3, P, H).transpose(1, 0, 2).reshape(P, 3 * H)).astype(np.float32)

    in_maps = []
    for c in range(NCORES):
        in_maps.append({
            "pred0": pred0,
            "fmsg": first_message.reshape(1, MSG).astype(np.float32),
            "wsT": WsT,
            "wmT": WmT,
            "bsrow": bs.reshape(1, H).astype(np.float32),
            "bmrow": bm.reshape(1, MSG).astype(np.float32),
            "sidx": np.ascontiguousarray(sched["sidx"][c]),
            "midx": np.ascontiguousarray(sched["midx"][c]),
            "fidx": np.ascontiguousarray(sched["fidx"][c]),
        })

    res = bass_utils.run_bass_kernel_spmd(
        nc, in_maps, core_ids=list(range(NCORES)))

    diag = np.zeros((N, H), np.float32)
    for c in range(NCORES):
        rs = sched["runs_of"][c]
        diag[rs] = res.results[c]["diag"][:len(rs)]

    logits = diag @ Wd.T + bd
    mx = logits.max(axis=-1, keepdims=True)
    z = logits - mx
    lse = np.log(np.exp(z).sum(axis=-1, keepdims=True))
    return (z - lse).astype(np.float32)
